# revision 44
# baseline (speedup 1.0000x reference)
"""Multi-head causal attention (bs=4, seq=2048, dm=1024, 16 heads) on 8 trn2 cores.

Sharding: core c = (batch b = c//2, head-group g = c%2). Each core computes
QKV projections for its batch restricted to its 8 heads, causal attention for
those heads, and a partial output projection (contracting its 512 z-columns
with the matching 512 rows of w_out). The host sums the two partials per
batch and transposes (each core returns out^T [dm, seq]).

v2 layout: all matmul operands bf16 (psum accumulation fp32). x arrives
pre-transposed from the host (xT [dm, seq]), eliminating on-device PE
transposes. Attention is software-pipelined: PV matmuls lag the S^T matmuls
by one k-chunk so the scalar-engine exp overlaps the PE. QKV-projection and
output-projection matmul "fill units" are interleaved into attention slots
to keep the PE busy while the act engine catches up. Causal masking via
affine_select narrowed to the 128-wide diagonal band. Output-projection
PSUM tiles are DMA'd straight to DRAM.
"""
import sys
sys.path.insert(0, "/opt/trn_rl_repo")

from collections import deque
from contextlib import ExitStack
from functools import partial

import numpy as np

import concourse.bass as bass
import concourse.tile as tile
from concourse import bacc, mybir
from concourse import bass_utils
from concourse.tile_rust import add_dep_helper

F32 = mybir.dt.float32
BF16 = mybir.dt.bfloat16
AF = mybir.ActivationFunctionType

BS, SEQ, DM, H, DK = 4, 2048, 1024, 16, 64
HL = 8          # heads per core
NPAIR = 4       # head pairs per core
N_CORES = 8
NQB = SEQ // 512    # 4 q blocks of 512
NKC = SEQ // 128    # 16 k chunks of 128
NDC = DM // 128     # 8 dm chunks

_CACHE = {}


def _build_nc(phases="ABC", repeat=1):
    nc = bacc.Bacc("TRN2", target_bir_lowering=False, debug=False,
                   enable_asserts=True, num_devices=N_CORES)

    xT_d = nc.dram_tensor("xT_b", [DM, SEQ], BF16, kind="ExternalInput").ap()
    wqk_d = nc.dram_tensor("w_qk", [DM, 1024], BF16, kind="ExternalInput").ap()
    wv_d = nc.dram_tensor("w_v", [DM, 512], BF16, kind="ExternalInput").ap()
    bqk_d = nc.dram_tensor("b_qk", [1024], F32, kind="ExternalInput").ap()
    bv_d = nc.dram_tensor("b_v", [512], BF16, kind="ExternalInput").ap()
    wo_d = nc.dram_tensor("w_out", [512, DM], BF16, kind="ExternalInput").ap()
    out_d = nc.dram_tensor("outT", [DM, SEQ], F32, kind="ExternalOutput").ap()

    def emit_once(tc, const, persist, pools):
        pp, psst, psz, pt_pool, zst_pool, rec_pool = pools

        bqk_sb = const.tile([128, 8], F32, tag="bqk")
        bv_bc = const.tile([128, 512], BF16, tag="bv")

        wqk_sb = persist.tile([128, NDC, 1024], BF16, tag="wqk")
        wv_sb = persist.tile([128, NDC, 512], BF16, tag="wv")
        wo_sb = persist.tile([128, 4, DM], BF16, tag="wo")
        xT_sb = persist.tile([128, NDC, SEQ], BF16, tag="xT")
        qkT = persist.tile([128, 8, SEQ], BF16, tag="qkT")   # 0-3 q pairs, 4-7 k
        v4 = persist.tile([128, NKC, HL, 65], BF16, tag="v4")
        zT = persist.tile([128, NPAIR, SEQ], BF16, tag="zT")

        nc.vector.memset(v4[:, :, :, 64:65], 1.0)

        # reciprocal expander: ones64.T @ rec broadcasts a [1, 512] denominator
        # reciprocal across 64 psum partitions (engine writes must start at a
        # 32-aligned partition, so each head gets its own expander matmul)
        ones64 = const.tile([1, 64], BF16, tag="ones64")
        nc.vector.memset(ones64[:], 1.0)

        # input DMAs, ordered so A(0) can start early: the first 256-col
        # xT/w_qk chunks arrive in d-chunk pairs matching the accumulation
        # order of qk_unit(0, 0), so its first matmul starts ~1.7us in.
        for dd in range(4):
            nc.sync.dma_start(
                xT_sb[:, 2 * dd:2 * dd + 2, 0:256],
                xT_d[256 * dd:256 * (dd + 1), 0:256]
                .rearrange("(c p) s -> p c s", p=128))
            nc.sync.dma_start(
                wqk_sb[:, 2 * dd:2 * dd + 2, 0:256],
                wqk_d[256 * dd:256 * (dd + 1), 0:256]
                .rearrange("(c p) n -> p c n", p=128))
        nc.sync.dma_start(
            xT_sb[:, :, 256:512],
            xT_d[:, 256:512].rearrange("(c p) s -> p c s", p=128))
        nc.sync.dma_start(
            wqk_sb[:, :, 256:512],
            wqk_d[:, 256:512].rearrange("(c p) n -> p c n", p=128))
        nc.sync.dma_start(bqk_sb[:], bqk_d.rearrange("(t p) -> p t", p=128))
        bv_src = bass.AP(tensor=bv_d.tensor, offset=bv_d.offset,
                         ap=[[0, 128]] + list(bv_d.ap))
        nc.sync.dma_start(bv_bc[:], bv_src)
        for h in range(2, 4):
            nc.sync.dma_start(
                wqk_sb[:, :, h * 256:(h + 1) * 256],
                wqk_d[:, h * 256:(h + 1) * 256].rearrange("(c p) n -> p c n", p=128))
        nc.sync.dma_start(wv_sb[:], wv_d.rearrange("(c p) n -> p c n", p=128))
        for qq in range(1, 4):
            nc.sync.dma_start(
                xT_sb[:, :, qq * 512:(qq + 1) * 512],
                xT_d[:, qq * 512:(qq + 1) * 512].rearrange("(c p) s -> p c s", p=128))
        nc.sync.dma_start(wo_sb[:], wo_d.rearrange("(c p) n -> p c n", p=128))

        # ---------------- unit generators (fill work) ----------------
        # Fill units yield every ~2 matmuls so the scheduler can interleave
        # ~0.4us slices of projection work between attention chunks, keeping
        # the PE busy while the act engine (the per-chunk straggler) catches
        # up.
        def qk_unit(qq, t):
            ps = pp.tile([128, 512], F32, tag="pp")
            # qq=0 runs during DMA warmup: halve the moving operand so the
            # first matmuls only need the first 256-col xT/w chunks.
            halves = 2 if qq == 0 else 1
            hw_ = 512 // halves
            for hh in range(halves):
                lo = qq * 512 + hh * hw_
                for d in range(NDC):
                    nc.tensor.matmul(ps[:, hh * hw_:(hh + 1) * hw_],
                                     wqk_sb[:, d, t * 128:(t + 1) * 128],
                                     xT_sb[:, d, lo:lo + hw_],
                                     start=(d == 0), stop=(d == NDC - 1))
                    if d % 2 == 1 and not (hh == halves - 1 and d == NDC - 1):
                        yield
            nc.vector.tensor_scalar_add(
                qkT[:, t, qq * 512:(qq + 1) * 512], ps[:], bqk_sb[:, t:t + 1])

        def v_unit(qq, s):
            kc = qq * 4 + s
            ps = pp.tile([128, 512], F32, tag="pp")
            for d in range(NDC):
                nc.tensor.matmul(ps[:], xT_sb[:, d, kc * 128:(kc + 1) * 128],
                                 wv_sb[:, d, :],
                                 start=(d == 0), stop=(d == NDC - 1))
                if d % 2 == 1 and d != NDC - 1:
                    yield
            nc.vector.tensor_add(
                v4[:, kc, :, 0:64],
                ps[:].rearrange("p (h e) -> p h e", h=HL),
                bv_bc[:].rearrange("p (h e) -> p h e", h=HL))

        def c_unit(s, t):
            po = pp.tile([128, 512], F32, tag="pp")
            for jj in range(NPAIR):
                nc.tensor.matmul(po[:], wo_sb[:, jj, t * 128:(t + 1) * 128],
                                 zT[:, jj, s * 512:(s + 1) * 512],
                                 start=(jj == 0), stop=(jj == NPAIR - 1))
                if jj == 1:
                    yield
            so = zst_pool.tile([128, 512], F32, tag="so")
            nc.vector.tensor_copy(so[:], po[:])
            nc.sync.dma_start(out_d[t * 128:(t + 1) * 128,
                                    s * 512:(s + 1) * 512], so[:])

        class MicroFill:
            def __init__(self):
                self.q = deque()
                self.cur = None   # (tag, running generator)

            def push(self, tag, genfn):
                self.q.append((tag, genfn))

            def step(self, n=1):
                for _ in range(n):
                    while True:
                        if self.cur is None:
                            if not self.q:
                                return
                            tag, fn = self.q.popleft()
                            self.cur = (tag, fn())
                        try:
                            next(self.cur[1])
                            break
                        except StopIteration:
                            self.cur = None

            def drain_tag(self, tag):
                if self.cur is not None and self.cur[0] == tag:
                    for _ in self.cur[1]:
                        pass
                    self.cur = None
                while self.q and self.q[0][0] == tag:
                    _, fn = self.q.popleft()
                    for _ in fn():
                        pass

            def drain_all(self):
                if self.cur is not None:
                    for _ in self.cur[1]:
                        pass
                    self.cur = None
                while self.q:
                    _, fn = self.q.popleft()
                    for _ in fn():
                        pass

        def a_block(qq):
            for t in range(8):
                for _ in qk_unit(qq, t):
                    pass
            for s in range(4):
                for _ in v_unit(qq, s):
                    pass

        # ---------------- attention ----------------
        # The S^T -> exp -> PV chain is pipelined with PV lagging two chunks
        # behind S (pvq holds the pending PV actions), flowing across pair
        # boundaries, so neither the exp latency nor psum-bank rotation gates
        # the PE.  Each pair's normalize runs in two deferred steps: the
        # reciprocal chain fires with its final (stop) PV action, and the
        # expander-matmul discharge lands one pair later via norm_hold.
        def finish_pair(qb, j, zA, zB, last):
            # softmax normalize: z / denom (denom = partition 64).  The psum
            # tile is staged to SBUF so its bank frees quickly; the last pair
            # has no successor, so it reads psum directly (shorter tail).
            recs, zsts = [], []
            for hidx, zh in ((0, zA), (1, zB)):
                zst = zst_pool.tile([65, 512], F32, tag="zst")
                nc.vector.tensor_copy(zst[:], zh[:])
                rec = rec_pool.tile([1, 512], BF16, tag="rec")
                with nc.allow_low_precision(reason="softmax denom in bf16"):
                    nc.vector.reciprocal(rec[:], zst[64:65, :])
                recs.append(rec)
                zsts.append(zst)

            def discharge(use_psst=False):
                # per head: expander matmul broadcasts rec to psum partitions
                # 0:64 (inputs of the multiply stay lane-aligned; only the
                # zT output is partition-shifted, the pattern the DVE
                # supports).  At the tail the attention psum pool is free, so
                # the expander borrows st slots instead of competing with the
                # C-unit rotation.
                for hidx in (0, 1):
                    if use_psst:
                        bct = psst.tile([128, 2, 512], F32, tag="st",
                                        name=f"bct{hidx}")
                        bc = bct[0:64, 0, :]
                    else:
                        bct = pp.tile([128, 512], F32, tag="pp",
                                      name=f"bct{hidx}")
                        bc = bct[0:64, :]
                    nc.tensor.matmul(bc, ones64[:], recs[hidx][:],
                                     start=True, stop=True)
                    nc.vector.tensor_mul(
                        zT[64 * hidx:64 * hidx + 64, j, qb * 512:(qb + 1) * 512],
                        zsts[hidx][0:64, :], bc)

            norm_hold.append(discharge)

        def b_pair(qb, j, fillq, norm_hold, pvq):
            nk = 4 * (qb + 1)
            zA = psz.tile([65, 512], F32, tag="z")
            zB = psz.tile([65, 512], F32, tag="z")
            last = (qb == NQB - 1 and j == NPAIR - 1)
            for kc in range(nk):
                if kc == 1:
                    while norm_hold:
                        norm_hold.popleft()()
                if kc % 2 == 1:
                    # ~0.4us of projection fill every other chunk covers the
                    # act engine's per-chunk deficit without delaying S
                    fillq.step()
                qoff = max(0, kc * 128 - qb * 512)  # diag narrowing
                stAB = psst.tile([128, 2, 512], F32, tag="st")
                nc.tensor.matmul(
                    stAB[:, 0, qoff:],
                    qkT[0:64, 4 + j, kc * 128:(kc + 1) * 128],
                    qkT[0:64, j, qb * 512 + qoff:(qb + 1) * 512],
                    start=True, stop=True)
                nc.tensor.matmul(
                    stAB[:, 1, qoff:],
                    qkT[64:128, 4 + j, kc * 128:(kc + 1) * 128],
                    qkT[64:128, j, qb * 512 + qoff:(qb + 1) * 512],
                    start=True, stop=True)
                ptAB = pt_pool.tile([128, 2, 512], BF16, tag="pt")
                nc.scalar.activation(ptAB[:, :, qoff:], stAB[:, :, qoff:],
                                     AF.Exp, scale=0.125)
                if kc >= 4 * qb:  # diagonal block: causal mask, 128-wide band
                    nc.gpsimd.affine_select(
                        out=ptAB[:, :, qoff:qoff + 128],
                        in_=ptAB[:, :, qoff:qoff + 128],
                        compare_op=mybir.AluOpType.is_ge, fill=0.0,
                        base=0, pattern=[[0, 2], [1, 128]],
                        channel_multiplier=-1)

                def pv_action(kc=kc, ptAB=ptAB, qoff=qoff, zA=zA, zB=zB,
                              j=j, stop=(kc == nk - 1)):
                    nc.tensor.matmul(zA[:, qoff:], v4[:, kc, 2 * j, :],
                                     ptAB[:, 0, qoff:],
                                     start=(kc == 0), stop=stop)
                    nc.tensor.matmul(zB[:, qoff:], v4[:, kc, 2 * j + 1, :],
                                     ptAB[:, 1, qoff:],
                                     start=(kc == 0), stop=stop)
                    if stop:
                        finish_pair(qb, j, zA, zB, last)

                pvq.append(pv_action)
                while len(pvq) > 3:
                    pvq.pop(0)()

        # ---------------- schedule ----------------
        a_block(0)
        fillq = MicroFill()
        for qq in (1, 2, 3):
            for t in range(8):
                fillq.push(f"A{qq}", partial(qk_unit, qq, t))
            for s in range(4):
                fillq.push(f"A{qq}", partial(v_unit, qq, s))

        norm_hold = deque()
        pvq = []
        pending_c = []
        if "B" in phases:
            for qb in range(NQB):
                fillq.drain_tag(f"A{qb}")
                for j in range(NPAIR):
                    b_pair(qb, j, fillq, norm_hold, pvq)
                    if j == 0 and pending_c:
                        # C(qb-1) units become poppable only after the first
                        # pair of B(qb) — its kc==1 slot discharged B(qb-1)'s
                        # final zT writes, so no PE stall (or deadlock) on them
                        for tag, fn in pending_c:
                            fillq.push(tag, fn)
                        pending_c = []
                if "C" in phases:
                    pending_c = [(f"C{qb}", partial(c_unit, qb, t))
                                 for t in range(8)]
        for act in pvq:
            act()
        pvq.clear()
        # leftover fill units (older C blocks) keep the PE busy while the
        # last pair's reciprocal chain completes
        fillq.drain_all()
        if pending_c:
            # C3 tail: jj=2/3 depend on the final pairs' zT discharges, so
            # each unit runs jj=0..1 up front and finishes (jj=2,3 + copy +
            # DMA) two units later, with the discharges emitted after the
            # first unit's independent matmuls.
            def finish_c3(t, po):
                for jj in (2, 3):
                    nc.tensor.matmul(po[:], wo_sb[:, jj, t * 128:(t + 1) * 128],
                                     zT[:, jj, 1536:2048],
                                     start=False, stop=(jj == 3))
                so = zst_pool.tile([128, 512], F32, tag="so")
                nc.vector.tensor_copy(so[:], po[:])
                nc.sync.dma_start(out_d[t * 128:(t + 1) * 128, 1536:2048],
                                  so[:])

            inflight = []
            for t in range(8):
                po = pp.tile([128, 512], F32, tag="pp")
                for jj in (0, 1):
                    nc.tensor.matmul(po[:], wo_sb[:, jj, t * 128:(t + 1) * 128],
                                     zT[:, jj, 1536:2048],
                                     start=(jj == 0), stop=False)
                inflight.append((t, po))
                if t == 1:
                    while norm_hold:
                        norm_hold.popleft()(use_psst=True)
                if len(inflight) == 2:
                    finish_c3(*inflight.pop(0))
            while inflight:
                finish_c3(*inflight.pop(0))
        while norm_hold:
            norm_hold.popleft()()

    with tile.TileContext(nc) as tc, ExitStack() as top:
        const = top.enter_context(tc.tile_pool(name="const", bufs=1))
        persist = top.enter_context(tc.tile_pool(name="persist", bufs=1))
        pp = top.enter_context(tc.tile_pool(name="pp", bufs=2, space="PSUM"))
        psst = top.enter_context(tc.tile_pool(name="psst", bufs=2, space="PSUM"))
        psz = top.enter_context(tc.tile_pool(name="psz", bufs=2, space="PSUM"))
        pt_pool = top.enter_context(tc.tile_pool(name="pt", bufs=6))
        zst_pool = top.enter_context(tc.tile_pool(name="zst", bufs=4))
        rec_pool = top.enter_context(tc.tile_pool(name="rec", bufs=4))
        pools = (pp, psst, psz, pt_pool, zst_pool, rec_pool)
        for _rep in range(repeat):
            emit_once(tc, const, persist, pools)

    nc.compile()
    return nc


def get_nc(phases="ABC", repeat=1):
    key = (phases, repeat)
    if key not in _CACHE:
        _CACHE[key] = _build_nc(phases, repeat)
    return _CACHE[key]


def make_in_maps(x, w_qkv, b_qkv, w_out):
    bf16 = np.dtype(mybir.dt.np(mybir.dt.bfloat16))
    x = np.asarray(x, dtype=np.float32)
    w_qkv = np.asarray(w_qkv, dtype=np.float32)
    b_qkv = np.asarray(b_qkv, dtype=np.float32)
    w_out = np.asarray(w_out, dtype=np.float32)
    in_maps = []
    for c in range(N_CORES):
        b, g = divmod(c, 2)
        cs = slice(512 * g, 512 * (g + 1))
        w_qk = np.ascontiguousarray(
            np.concatenate([w_qkv[:, cs],
                            w_qkv[:, 1024 + 512 * g:1024 + 512 * (g + 1)]],
                           axis=1).astype(bf16))
        w_v = np.ascontiguousarray(
            w_qkv[:, 2048 + 512 * g:2048 + 512 * (g + 1)].astype(bf16))
        b_qk = np.ascontiguousarray(
            np.concatenate([b_qkv[cs], b_qkv[1024 + 512 * g:1024 + 512 * (g + 1)]]))
        b_v = np.ascontiguousarray(
            b_qkv[2048 + 512 * g:2048 + 512 * (g + 1)].astype(bf16))
        w_o = np.ascontiguousarray(w_out[512 * g:512 * (g + 1), :].astype(bf16))
        xT = np.ascontiguousarray(x[b].T.astype(bf16))
        in_maps.append({
            "xT_b": xT, "w_qk": w_qk, "w_v": w_v, "b_qk": b_qk,
            "b_v": b_v, "w_out": w_o,
        })
    return in_maps


def gather_output(results, b_out):
    b_out = np.asarray(b_out, dtype=np.float32)
    outs = []
    for b in range(BS):
        pT = results[2 * b]["outT"] + results[2 * b + 1]["outT"]  # [dm, seq]
        outs.append(pT.T + b_out[None, :])
    return np.stack(outs).astype(np.float32)


def kernel(x, w_qkv, b_qkv, w_out, b_out):
    nc = get_nc()
    in_maps = make_in_maps(x, w_qkv, b_qkv, w_out)
    res = bass_utils.run_bass_kernel_spmd(nc, in_maps,
                                          core_ids=list(range(N_CORES)))
    return gather_output(res.results, b_out)


# revision 47
# speedup vs baseline: 1.0180x; 1.0180x over previous
"""Multi-head causal attention (bs=4, seq=2048, dm=1024, 16 heads) on 8 trn2 cores.

Sharding: core c = (batch b = c//2, head-group g = c%2). Each core computes
QKV projections for its batch restricted to its 8 heads, causal attention for
those heads, and a partial output projection (contracting its 512 z-columns
with the matching 512 rows of w_out). The host sums the two partials per
batch and transposes (each core returns out^T [dm, seq]).

v2 layout: all matmul operands bf16 (psum accumulation fp32). x arrives
pre-transposed from the host (xT [dm, seq]), eliminating on-device PE
transposes. Attention is software-pipelined: PV matmuls lag the S^T matmuls
by one k-chunk so the scalar-engine exp overlaps the PE. QKV-projection and
output-projection matmul "fill units" are interleaved into attention slots
to keep the PE busy while the act engine catches up. Causal masking via
affine_select narrowed to the 128-wide diagonal band. Output-projection
PSUM tiles are DMA'd straight to DRAM.
"""
import sys
sys.path.insert(0, "/opt/trn_rl_repo")

from collections import deque
from contextlib import ExitStack
from functools import partial

import numpy as np

import concourse.bass as bass
import concourse.tile as tile
from concourse import bacc, mybir
from concourse import bass_utils
from concourse.tile_rust import add_dep_helper

F32 = mybir.dt.float32
BF16 = mybir.dt.bfloat16
AF = mybir.ActivationFunctionType

BS, SEQ, DM, H, DK = 4, 2048, 1024, 16, 64
HL = 8          # heads per core
NPAIR = 4       # head pairs per core
N_CORES = 8
NQB = SEQ // 512    # 4 q blocks of 512
NKC = SEQ // 128    # 16 k chunks of 128
NDC = DM // 128     # 8 dm chunks

_CACHE = {}


def _build_nc(phases="ABC", repeat=1):
    nc = bacc.Bacc("TRN2", target_bir_lowering=False, debug=False,
                   enable_asserts=True, num_devices=N_CORES)

    xT_d = nc.dram_tensor("xT_b", [DM, SEQ], BF16, kind="ExternalInput").ap()
    wqk_d = nc.dram_tensor("w_qk", [DM, 1024], BF16, kind="ExternalInput").ap()
    wv_d = nc.dram_tensor("w_v", [DM, 512], BF16, kind="ExternalInput").ap()
    bqk_d = nc.dram_tensor("b_qk", [1024], F32, kind="ExternalInput").ap()
    bv_d = nc.dram_tensor("b_v", [512], BF16, kind="ExternalInput").ap()
    wo_d = nc.dram_tensor("w_out", [512, DM], BF16, kind="ExternalInput").ap()
    out_d = nc.dram_tensor("outT", [DM, SEQ], F32, kind="ExternalOutput").ap()

    # per-(qb, pair, head) scratch for softmax-denominator broadcast bounce
    scratch = nc.dram_tensor("scratch", [NQB, NPAIR, 2, 512], BF16).ap()

    def emit_once(tc, const, persist, pools):
        pp, psst, psz, pt_pool, zst_pool, rec_pool = pools

        bqk_sb = const.tile([128, 8], F32, tag="bqk")
        bv_bc = const.tile([128, 512], BF16, tag="bv")

        wqk_sb = persist.tile([128, NDC, 1024], BF16, tag="wqk")
        wv_sb = persist.tile([128, NDC, 512], BF16, tag="wv")
        wo_sb = persist.tile([128, 4, DM], BF16, tag="wo")
        xT_sb = persist.tile([128, NDC, SEQ], BF16, tag="xT")
        qkT = persist.tile([128, 8, SEQ], BF16, tag="qkT")   # 0-3 q pairs, 4-7 k
        v4 = persist.tile([128, NKC, HL, 65], BF16, tag="v4")
        zT = persist.tile([128, NPAIR, SEQ], BF16, tag="zT")

        nc.vector.memset(v4[:, :, :, 64:65], 1.0)



        # input DMAs, ordered so A(0) can start early: the first 256-col
        # xT/w_qk chunks arrive in d-chunk pairs matching the accumulation
        # order of qk_unit(0, 0), so its first matmul starts ~1.7us in.
        for dd in range(4):
            nc.sync.dma_start(
                xT_sb[:, 2 * dd:2 * dd + 2, 0:256],
                xT_d[256 * dd:256 * (dd + 1), 0:256]
                .rearrange("(c p) s -> p c s", p=128))
            nc.sync.dma_start(
                wqk_sb[:, 2 * dd:2 * dd + 2, 0:256],
                wqk_d[256 * dd:256 * (dd + 1), 0:256]
                .rearrange("(c p) n -> p c n", p=128))
        nc.sync.dma_start(
            xT_sb[:, :, 256:512],
            xT_d[:, 256:512].rearrange("(c p) s -> p c s", p=128))
        nc.sync.dma_start(
            wqk_sb[:, :, 256:512],
            wqk_d[:, 256:512].rearrange("(c p) n -> p c n", p=128))
        nc.sync.dma_start(bqk_sb[:], bqk_d.rearrange("(t p) -> p t", p=128))
        bv_src = bass.AP(tensor=bv_d.tensor, offset=bv_d.offset,
                         ap=[[0, 128]] + list(bv_d.ap))
        nc.sync.dma_start(bv_bc[:], bv_src)
        for h in range(2, 4):
            nc.sync.dma_start(
                wqk_sb[:, :, h * 256:(h + 1) * 256],
                wqk_d[:, h * 256:(h + 1) * 256].rearrange("(c p) n -> p c n", p=128))
        nc.sync.dma_start(wv_sb[:], wv_d.rearrange("(c p) n -> p c n", p=128))
        for qq in range(1, 4):
            nc.sync.dma_start(
                xT_sb[:, :, qq * 512:(qq + 1) * 512],
                xT_d[:, qq * 512:(qq + 1) * 512].rearrange("(c p) s -> p c s", p=128))
        nc.sync.dma_start(wo_sb[:], wo_d.rearrange("(c p) n -> p c n", p=128))

        # ---------------- unit generators (fill work) ----------------
        # Fill units yield every ~2 matmuls so the scheduler can interleave
        # ~0.4us slices of projection work between attention chunks, keeping
        # the PE busy while the act engine (the per-chunk straggler) catches
        # up.
        def qk_unit(qq, t):
            ps = pp.tile([128, 512], F32, tag="pp")
            # qq=0 runs during DMA warmup: halve the moving operand so the
            # first matmuls only need the first 256-col xT/w chunks.
            halves = 2 if qq == 0 else 1
            hw_ = 512 // halves
            for hh in range(halves):
                lo = qq * 512 + hh * hw_
                for d in range(NDC):
                    nc.tensor.matmul(ps[:, hh * hw_:(hh + 1) * hw_],
                                     wqk_sb[:, d, t * 128:(t + 1) * 128],
                                     xT_sb[:, d, lo:lo + hw_],
                                     start=(d == 0), stop=(d == NDC - 1))
                    if d % 2 == 1 and not (hh == halves - 1 and d == NDC - 1):
                        yield
            nc.vector.tensor_scalar_add(
                qkT[:, t, qq * 512:(qq + 1) * 512], ps[:], bqk_sb[:, t:t + 1])

        def v_unit(qq, s):
            kc = qq * 4 + s
            ps = pp.tile([128, 512], F32, tag="pp")
            for d in range(NDC):
                nc.tensor.matmul(ps[:], xT_sb[:, d, kc * 128:(kc + 1) * 128],
                                 wv_sb[:, d, :],
                                 start=(d == 0), stop=(d == NDC - 1))
                if d % 2 == 1 and d != NDC - 1:
                    yield
            nc.vector.tensor_add(
                v4[:, kc, :, 0:64],
                ps[:].rearrange("p (h e) -> p h e", h=HL),
                bv_bc[:].rearrange("p (h e) -> p h e", h=HL))

        def c_unit(s, t):
            po = pp.tile([128, 512], F32, tag="pp")
            for jj in range(NPAIR):
                nc.tensor.matmul(po[:], wo_sb[:, jj, t * 128:(t + 1) * 128],
                                 zT[:, jj, s * 512:(s + 1) * 512],
                                 start=(jj == 0), stop=(jj == NPAIR - 1))
                if jj == 1:
                    yield
            so = zst_pool.tile([128, 512], F32, tag="so")
            nc.vector.tensor_copy(so[:], po[:])
            nc.sync.dma_start(out_d[t * 128:(t + 1) * 128,
                                    s * 512:(s + 1) * 512], so[:])

        class MicroFill:
            def __init__(self):
                self.q = deque()
                self.cur = None   # (tag, running generator)

            def push(self, tag, genfn):
                self.q.append((tag, genfn))

            def step(self, n=1):
                for _ in range(n):
                    while True:
                        if self.cur is None:
                            if not self.q:
                                return
                            tag, fn = self.q.popleft()
                            self.cur = (tag, fn())
                        try:
                            next(self.cur[1])
                            break
                        except StopIteration:
                            self.cur = None

            def drain_tag(self, tag):
                if self.cur is not None and self.cur[0] == tag:
                    for _ in self.cur[1]:
                        pass
                    self.cur = None
                while self.q and self.q[0][0] == tag:
                    _, fn = self.q.popleft()
                    for _ in fn():
                        pass

            def drain_all(self):
                if self.cur is not None:
                    for _ in self.cur[1]:
                        pass
                    self.cur = None
                while self.q:
                    _, fn = self.q.popleft()
                    for _ in fn():
                        pass

        def a_block(qq):
            for t in range(8):
                for _ in qk_unit(qq, t):
                    pass
            for s in range(4):
                for _ in v_unit(qq, s):
                    pass

        # ---------------- attention ----------------
        # The S^T -> exp -> PV chain is pipelined with PV lagging two chunks
        # behind S (pvq holds the pending PV actions), flowing across pair
        # boundaries, so neither the exp latency nor psum-bank rotation gates
        # the PE.  Each pair's normalize runs in two deferred steps: the
        # reciprocal chain fires with its final (stop) PV action, and the
        # expander-matmul discharge lands one pair later via norm_hold.
        def finish_pair(qb, j, zA, zB, last):
            # softmax normalize: z / denom (denom = partition 64).  The psum
            # tile is staged to SBUF so its bank frees quickly; the last pair
            # has no successor, so it reads psum directly (shorter tail).
            bcs, zsts = [], []
            for hidx, zh in ((0, zA), (1, zB)):
                zst = zst_pool.tile([65, 512], F32, tag="zst")
                nc.vector.tensor_copy(zst[:], zh[:])
                rec = rec_pool.tile([1, 512], BF16, tag="rec")
                with nc.allow_low_precision(reason="softmax denom in bf16"):
                    nc.vector.reciprocal(rec[:], zst[64:65, :])
                # broadcast 1/denom across 64 partitions via a DRAM bounce
                # (DMA can replicate partitions; engines cannot)
                sc = scratch[qb, j, hidx, :]
                w_i = nc.sync.dma_start(sc, rec[:])
                bc = rec_pool.tile([64, 512], BF16, tag="bc")
                bc_src = bass.AP(tensor=sc.tensor, offset=sc.offset,
                                 ap=[[0, 64]] + list(sc.ap))
                r_i = nc.sync.dma_start(bc[:], bc_src)
                add_dep_helper(r_i.ins, w_i.ins, sync=True,
                               reason="denominator bounce RAW")
                bcs.append(bc)
                zsts.append(zst)

            def discharge(use_psst=False):
                for hidx in (0, 1):
                    nc.vector.tensor_mul(
                        zT[64 * hidx:64 * hidx + 64, j, qb * 512:(qb + 1) * 512],
                        zsts[hidx][0:64, :], bcs[hidx][:])

            norm_hold.append(discharge)

        def b_pair(qb, j, fillq, norm_hold, pvq):
            nk = 4 * (qb + 1)
            zA = psz.tile([65, 512], F32, tag="z")
            zB = psz.tile([65, 512], F32, tag="z")
            last = (qb == NQB - 1 and j == NPAIR - 1)
            for kc in range(nk):
                if kc == 1:
                    while norm_hold:
                        norm_hold.popleft()()
                if kc % 2 == 1:
                    # ~0.4us of projection fill every other chunk covers the
                    # act engine's per-chunk deficit without delaying S
                    fillq.step()
                qoff = max(0, kc * 128 - qb * 512)  # diag narrowing
                stAB = psst.tile([128, 2, 512], F32, tag="st")
                nc.tensor.matmul(
                    stAB[:, 0, qoff:],
                    qkT[0:64, 4 + j, kc * 128:(kc + 1) * 128],
                    qkT[0:64, j, qb * 512 + qoff:(qb + 1) * 512],
                    start=True, stop=True)
                nc.tensor.matmul(
                    stAB[:, 1, qoff:],
                    qkT[64:128, 4 + j, kc * 128:(kc + 1) * 128],
                    qkT[64:128, j, qb * 512 + qoff:(qb + 1) * 512],
                    start=True, stop=True)
                ptAB = pt_pool.tile([128, 2, 512], BF16, tag="pt")
                nc.scalar.activation(ptAB[:, :, qoff:], stAB[:, :, qoff:],
                                     AF.Exp, scale=0.125)
                if kc >= 4 * qb:  # diagonal block: causal mask, 128-wide band
                    nc.gpsimd.affine_select(
                        out=ptAB[:, :, qoff:qoff + 128],
                        in_=ptAB[:, :, qoff:qoff + 128],
                        compare_op=mybir.AluOpType.is_ge, fill=0.0,
                        base=0, pattern=[[0, 2], [1, 128]],
                        channel_multiplier=-1)

                def pv_action(kc=kc, ptAB=ptAB, qoff=qoff, zA=zA, zB=zB,
                              j=j, stop=(kc == nk - 1)):
                    nc.tensor.matmul(zA[:, qoff:], v4[:, kc, 2 * j, :],
                                     ptAB[:, 0, qoff:],
                                     start=(kc == 0), stop=stop)
                    nc.tensor.matmul(zB[:, qoff:], v4[:, kc, 2 * j + 1, :],
                                     ptAB[:, 1, qoff:],
                                     start=(kc == 0), stop=stop)
                    if stop:
                        finish_pair(qb, j, zA, zB, last)

                pvq.append(pv_action)
                while len(pvq) > 3:
                    pvq.pop(0)()

        # ---------------- schedule ----------------
        a_block(0)
        fillq = MicroFill()
        for qq in (1, 2, 3):
            for t in range(8):
                fillq.push(f"A{qq}", partial(qk_unit, qq, t))
            for s in range(4):
                fillq.push(f"A{qq}", partial(v_unit, qq, s))

        norm_hold = deque()
        pvq = []
        pending_c = []
        if "B" in phases:
            for qb in range(NQB):
                fillq.drain_tag(f"A{qb}")
                for j in range(NPAIR):
                    b_pair(qb, j, fillq, norm_hold, pvq)
                    if j == 0 and pending_c:
                        # C(qb-1) units become poppable only after the first
                        # pair of B(qb) — its kc==1 slot discharged B(qb-1)'s
                        # final zT writes, so no PE stall (or deadlock) on them
                        for tag, fn in pending_c:
                            fillq.push(tag, fn)
                        pending_c = []
                if "C" in phases:
                    pending_c = [(f"C{qb}", partial(c_unit, qb, t))
                                 for t in range(8)]
        for act in pvq:
            act()
        pvq.clear()
        # leftover fill units (older C blocks) keep the PE busy while the
        # last pair's reciprocal chain completes
        fillq.drain_all()
        if pending_c:
            # C3 tail: jj=2/3 depend on the final pairs' zT discharges, so
            # each unit runs jj=0..1 up front and finishes (jj=2,3 + copy +
            # DMA) two units later, with the discharges emitted after the
            # first unit's independent matmuls.
            def finish_c3(t, po):
                for jj in (2, 3):
                    nc.tensor.matmul(po[:], wo_sb[:, jj, t * 128:(t + 1) * 128],
                                     zT[:, jj, 1536:2048],
                                     start=False, stop=(jj == 3))
                so = zst_pool.tile([128, 512], F32, tag="so")
                nc.vector.tensor_copy(so[:], po[:])
                nc.sync.dma_start(out_d[t * 128:(t + 1) * 128, 1536:2048],
                                  so[:])

            inflight = []
            for t in range(8):
                po = pp.tile([128, 512], F32, tag="pp")
                for jj in (0, 1):
                    nc.tensor.matmul(po[:], wo_sb[:, jj, t * 128:(t + 1) * 128],
                                     zT[:, jj, 1536:2048],
                                     start=(jj == 0), stop=False)
                inflight.append((t, po))
                if t == 1:
                    while norm_hold:
                        norm_hold.popleft()(use_psst=True)
                if len(inflight) == 2:
                    finish_c3(*inflight.pop(0))
            while inflight:
                finish_c3(*inflight.pop(0))
        while norm_hold:
            norm_hold.popleft()()

    with tile.TileContext(nc) as tc, ExitStack() as top:
        const = top.enter_context(tc.tile_pool(name="const", bufs=1))
        persist = top.enter_context(tc.tile_pool(name="persist", bufs=1))
        pp = top.enter_context(tc.tile_pool(name="pp", bufs=2, space="PSUM"))
        psst = top.enter_context(tc.tile_pool(name="psst", bufs=2, space="PSUM"))
        psz = top.enter_context(tc.tile_pool(name="psz", bufs=2, space="PSUM"))
        pt_pool = top.enter_context(tc.tile_pool(name="pt", bufs=6))
        zst_pool = top.enter_context(tc.tile_pool(name="zst", bufs=4))
        rec_pool = top.enter_context(tc.tile_pool(name="rec", bufs=4))
        pools = (pp, psst, psz, pt_pool, zst_pool, rec_pool)
        for _rep in range(repeat):
            emit_once(tc, const, persist, pools)

    nc.compile()
    return nc


def get_nc(phases="ABC", repeat=1):
    key = (phases, repeat)
    if key not in _CACHE:
        _CACHE[key] = _build_nc(phases, repeat)
    return _CACHE[key]


def make_in_maps(x, w_qkv, b_qkv, w_out):
    bf16 = np.dtype(mybir.dt.np(mybir.dt.bfloat16))
    x = np.asarray(x, dtype=np.float32)
    w_qkv = np.asarray(w_qkv, dtype=np.float32)
    b_qkv = np.asarray(b_qkv, dtype=np.float32)
    w_out = np.asarray(w_out, dtype=np.float32)
    in_maps = []
    for c in range(N_CORES):
        b, g = divmod(c, 2)
        cs = slice(512 * g, 512 * (g + 1))
        w_qk = np.ascontiguousarray(
            np.concatenate([w_qkv[:, cs],
                            w_qkv[:, 1024 + 512 * g:1024 + 512 * (g + 1)]],
                           axis=1).astype(bf16))
        w_v = np.ascontiguousarray(
            w_qkv[:, 2048 + 512 * g:2048 + 512 * (g + 1)].astype(bf16))
        b_qk = np.ascontiguousarray(
            np.concatenate([b_qkv[cs], b_qkv[1024 + 512 * g:1024 + 512 * (g + 1)]]))
        b_v = np.ascontiguousarray(
            b_qkv[2048 + 512 * g:2048 + 512 * (g + 1)].astype(bf16))
        w_o = np.ascontiguousarray(w_out[512 * g:512 * (g + 1), :].astype(bf16))
        xT = np.ascontiguousarray(x[b].T.astype(bf16))
        in_maps.append({
            "xT_b": xT, "w_qk": w_qk, "w_v": w_v, "b_qk": b_qk,
            "b_v": b_v, "w_out": w_o,
        })
    return in_maps


def gather_output(results, b_out):
    b_out = np.asarray(b_out, dtype=np.float32)
    outs = []
    for b in range(BS):
        pT = results[2 * b]["outT"] + results[2 * b + 1]["outT"]  # [dm, seq]
        outs.append(pT.T + b_out[None, :])
    return np.stack(outs).astype(np.float32)


def kernel(x, w_qkv, b_qkv, w_out, b_out):
    nc = get_nc()
    in_maps = make_in_maps(x, w_qkv, b_qkv, w_out)
    res = bass_utils.run_bass_kernel_spmd(nc, in_maps,
                                          core_ids=list(range(N_CORES)))
    return gather_output(res.results, b_out)
